# revision 1
# baseline (speedup 1.0000x reference)
"""Multi-head self-attention (B=4, T=2048, D=1024, H=16) on 8 TRN2 NeuronCores.

Reference quirk: softmax normalizes over the QUERY axis (dim=2 of
[B,H,T1,T2]), i.e. attn[q,k] = exp(s[q,k]) / sum_q' exp(s[q',k]).

Sharding (fully SPMD, one NEFF for all 8 cores):
  core c -> batch b = c//2, head-group g = c%2 (8 heads = 512 cols of Wq/Wk/Wv).
  Host pre-slices AND pre-transposes per-core inputs (xT, wqT/wkT/wvT), runs
  the kernel, and stitches the 8 transposed [E, T] output shards back.

Device algorithm per core:
  1. QT/KT [128e, T] per head-pair (partition = head dims of 2 heads),
     V [128t, 512e] natural, via PE from xT / w*T tiles (fp32r).
  2. Per head-pair, per 128-wide key chunk:
       S' = K @ Q^T chunk [128 k, T q] in PSUM (row-tiled pair: head A rows
       0-63, head B rows 64-127, concurrent).
       exp: 3 of 4 [128,1024] tiles via ScalarE ACT (accum_out = Z partials);
       the 4th tile (head A, qb0) via the Vector engine: Schraudolph bit-trick
       (tensor_scalar fp32->int32 round-convert) + a custom 7-stage DVE op
       that polynomial-corrects the mantissa (max rel err ~0.54%) and
       accumulates the Z partial. This offloads ~25% of the exp stream from
       the Scalar engine (the serial bottleneck).
       V'[k,:] = V[k,:] / Z[k] (Z partials summed on gpsimd, reciprocal on
       VectorE, fold into a zero-padded [128,128] V' pair tile),
       outT[d, q] += matmul: lhsT=V'_pad, rhs=P [128k, 512q], PSUM-accumulated.
  3. Epilogue: acc -> SBUF -> DRAM as outT [E, T]; final transpose on host.

Pipelining: AV is deferred one chunk (so it never waits on the Z chain),
qb1's B-tile scores are emitted first (their PSUM buf frees via the fast DVE
path), and projections fill the PE between score bursts with staggered
deadlines (ramp covers chunk-0-critical pieces and pair 1's QT).
"""

import operator

import numpy as np

B, T, D, H = 4, 2048, 1024, 16
DH = D // H
SCALE = 1.0 / (DH**0.5)
N_CORES = 8
E = D // 2  # 512 output cols per core (8 heads)
N_PAIRS = 4  # head-pairs per core
N_DC = D // 128  # 8 contraction chunks for projections
N_KC = T // 128  # 16 key chunks
QB = 1024  # exp free-dim block (2 PSUM banks)

# Schraudolph exp constants: z = round(2^23 * (log2e*SCALE*s + 127)),
# bitcast to fp32 gives 2^i*(1+f); custom DVE op multiplies by
# (1 + QC*f*(f-1)) to correct the mantissa interpolation.
A_SCHRAUD = float((2.0**23) * np.log2(np.e) * SCALE)
B_SCHRAUD = 127.0 * (2.0**23)
MASK_VAL = float(np.int32(0x007FFFFF).view(np.float32))  # +subnormal mantissa mask
TWO_P126 = float(np.float32(2.0**126))
QC = 0.23547743862603948

_built = None  # (nc,) cache so repeat kernel() calls skip rebuild/recompile
_exp_op = None


def _np_reference(x, padding_mask, Wq, Wk, Wv):
    """Pure-numpy fallback, used only if the mask is not all-ones."""
    x64 = x.astype(np.float64)
    Q = (x64 @ Wq.T.astype(np.float64)).reshape(B, T, H, DH).transpose(0, 2, 1, 3)
    K = (x64 @ Wk.T.astype(np.float64)).reshape(B, T, H, DH).transpose(0, 2, 1, 3)
    V = (x64 @ Wv.T.astype(np.float64)).reshape(B, T, H, DH).transpose(0, 2, 1, 3)
    s = np.einsum("bhqd,bhkd->bhqk", Q, K) * SCALE
    s = np.where(padding_mask[:, None, :, :] == 0, -np.inf, s)
    s = s - s.max(axis=2, keepdims=True)
    p = np.exp(s)
    p = p / p.sum(axis=2, keepdims=True)
    out = np.einsum("bhqk,bhkd->bhqd", p, V)
    return out.transpose(0, 2, 1, 3).reshape(B, T, D).astype(np.float32)


def _get_exp_op():
    """Register (once) the custom DVE op: out = z0*(1 + QC*f*(f-1)) with
    f extracted from z0's mantissa bits, plus a running sum (accum_out)."""
    global _exp_op
    if _exp_op is not None:
        return _exp_op
    import concourse.dve_ops as dve_ops_mod
    from concourse.dve_ops import DveOp
    from concourse.dve_spec import AluOp, Bin, C0, C1, C2, One, Spec, Src0, Zero, lower
    from concourse.dve_uop import DveOpSpec

    _a = Bin(AluOp.BITWISE_AND, Src0, C0)  # mantissa bits as +subnormal
    _d = _a * C1  # f in [0,1)  (subnormal * 2^126)
    _e = _d - One
    _u = _d * _e
    _v = _u * C2
    _t = _v * Src0
    spec = Spec(body=Src0 + _t, accum=operator.add, accum_init=Zero)

    name = "EXP_SFIX_ANT"
    existing = next((o for o in dve_ops_mod.OPS if o.name == name), None)
    if existing is not None:
        _exp_op = existing
        return existing
    sha = DveOpSpec(name=name, opcode=0, uops=lower(spec, ver="v3"), rd1_en=False).sha(
        "v3"
    )
    op = DveOp(name, spec, subdim=False, uops_sha={"v3": sha})
    dve_ops_mod.OPS.append(op)
    dve_ops_mod._SUB_OPCODE_FOR_NAME[name] = (
        dve_ops_mod._CUSTOM_DVE_ROW_BASE + len(dve_ops_mod.OPS) - 1
    )
    _exp_op = op
    return op


def _split_multi_waits(nc):
    """Walrus caps sync waits at 1 per instruction; Tile's tail drain can carry
    several. Move the extras onto single-wait drains appended to the previous
    basic block (same engine, earlier in program order)."""
    import concourse.mybir as mybir

    blocks = list(nc.m.functions[0].blocks)
    for bi, blk in enumerate(blocks):
        for inst in blk.instructions:
            if type(inst).__name__ not in ("InstDrain", "InstNoOp", "InstEventSemaphore"):
                continue
            si = inst.sync_info
            if si is not None and si.on_wait and len(si.on_wait) > 1:
                waits = list(si.on_wait)
                keep, extra = waits[-1], waits[:-1]
                assert all(w.wait_mode == "sem-ge-imm" for w in extra), extra
                si.on_wait = [keep]
                assert bi > 0, "multi-wait in first block"
                prev = blocks[bi - 1]
                for j, w in enumerate(extra):
                    d = mybir.InstDrain(
                        name=f"{inst.name}-ws{j}",
                        engine=inst.engine,
                        sync_info=mybir.SyncInfo(on_wait=[w], on_update=[]),
                    )
                    prev.add_instruction(d)


def _build_kernel(tc, xT, wqT, wkT, wvT, outT):
    import concourse.bass as bass  # noqa: F401
    import concourse.mybir as mybir

    nc = tc.nc
    FP = mybir.dt.float32
    FR = mybir.dt.float32r
    BF = mybir.dt.bfloat16
    I32 = mybir.dt.int32
    Exp = mybir.ActivationFunctionType.Exp
    exp_op = _get_exp_op()

    # long-lived pools
    xw = tc.alloc_tile_pool(name="xw", bufs=1)
    wp = tc.alloc_tile_pool(name="wp", bufs=3)
    qkv = tc.alloc_tile_pool(name="qkv", bufs=1)
    # PSUM: S pool (2x [128,QB] = 4 banks) shared by scores and projections;
    # acc pool (2x [128,QB] = 4 banks) for outT accumulation.
    sps = tc.alloc_tile_pool(name="sps", bufs=2, space="PSUM")
    accps = tc.alloc_tile_pool(name="accps", bufs=1, space="PSUM")
    pp = tc.alloc_tile_pool(name="pp", bufs=12)
    zp = tc.alloc_tile_pool(name="zp", bufs=6)
    z0p = tc.alloc_tile_pool(name="z0p", bufs=2)
    vpp = tc.alloc_tile_pool(name="vpp", bufs=6)
    op = tc.alloc_tile_pool(name="op", bufs=2)

    # ---- loads ----
    # Weights first (small), then x in query-quarter-major order: the first
    # ramp pieces (KT0-tt0, QT0-tt0) need only quarter tt=0 of every xT chunk,
    # so they complete at ~25% of the x stream instead of after all of it.
    xTs = [None] * N_DC
    wq, wk, wv = [None] * N_DC, [None] * N_DC, [None] * N_DC
    for dc in range(N_DC):
        xTs[dc] = xw.tile([128, T], FR, name=f"xT{dc}", tag=f"x{dc}")
        for ws, wap, label in ((wq, wqT, "wq"), (wk, wkT, "wk"), (wv, wvT, "wv")):
            wt = wp.tile([128, E], FR, name=f"{label}{dc}", tag=f"w{dc}")
            nc.sync.dma_start(out=wt, in_=wap[dc * 128 : (dc + 1) * 128, :])
            ws[dc] = wt
    for tt in range(4):
        for dc in range(N_DC):
            nc.sync.dma_start(
                out=xTs[dc][:, tt * 512 : (tt + 1) * 512],
                in_=xT[dc * 128 : (dc + 1) * 128, tt * 512 : (tt + 1) * 512],
            )

    # ---- projection emitters (psum borrowed from the S pool tag) ----
    copy_flip = [0]
    ramp = [True]  # during the upfront ramp ScalarE is idle; share copies

    def _proj_copy(dst, src):
        """PSUM->SBUF projection copies: alternate ScalarE/VectorE during the
        upfront ramp (ACT idle there); VectorE only in steady state (ACT is
        the bottleneck engine then)."""
        if ramp[0] and copy_flip[0] % 2 == 0:
            nc.scalar.copy(dst, src)
        else:
            nc.vector.tensor_copy(dst, src)
        copy_flip[0] += 1

    def project_eT_tile(ws, pair, tt, et):
        """One [128, 512] t-block of QT/KT pair tile `et` (bf16 [128, T])."""
        ps = sps.tile([128, QB], FP, name=f"ps_{et.tensor.name}_{tt}", tag="s")
        for dc in range(N_DC):
            nc.tensor.matmul(
                ps[:, 0:512],
                ws[dc][:, pair * 128 : (pair + 1) * 128],
                xTs[dc][:, tt * 512 : (tt + 1) * 512],
                start=(dc == 0),
                stop=(dc == N_DC - 1),
            )
        _proj_copy(et[:, tt * 512 : (tt + 1) * 512], ps[:, 0:512])

    def project_v_tile(tt):
        v = qkv.tile([128, E], BF, name=f"v{tt}", tag=f"v{tt}")
        ps = sps.tile([128, QB], FP, name=f"ps_v{tt}", tag="s")
        for dc in range(N_DC):
            nc.tensor.matmul(
                ps[:, 0:512],
                xTs[dc][:, tt * 128 : (tt + 1) * 128],
                wv[dc],
                start=(dc == 0),
                stop=(dc == N_DC - 1),
            )
        _proj_copy(v, ps[:, 0:512])
        return v

    QT = [None] * N_PAIRS
    KT = [None] * N_PAIRS
    V = [None] * N_KC

    # pair-0 QT/KT + V[0] upfront (ramp); V[c] and later pairs' QT/KT spread
    # into the chunk stream as PE fillers (keeps the HAM clock gate warm).
    for pair in range(N_PAIRS):
        QT[pair] = qkv.tile([128, T], BF, name=f"qt{pair}", tag=f"qt{pair}")
        KT[pair] = qkv.tile([128, T], BF, name=f"kt{pair}", tag=f"kt{pair}")
    # Ramp (DMA-gated): chunk-0-critical pieces first (KT0-tt0, QT0, V0),
    # then pair-1's early-deadline pieces. Late-deadline pieces (KT tails:
    # KT[p] piece tt isn't needed until pair p's chunk 4*tt) spread into the
    # chunk stream as PE fillers with per-pair schedules below.
    project_eT_tile(wk, 0, 0, KT[0])
    for tt in range(4):
        project_eT_tile(wq, 0, tt, QT[0])
    V[0] = project_v_tile(0)
    V[1] = project_v_tile(1)
    for tt in range(4):
        project_eT_tile(wq, 1, tt, QT[1])
    project_eT_tile(wk, 1, 0, KT[1])
    ramp[0] = False

    def emit_filler(p, c):
        if p == 0:
            if c < 3:  # KT0 piece tt (needed by chunk 4*tt)
                project_eT_tile(wk, 0, c + 1, KT[0])
            if c + 2 < N_KC:
                V[c + 2] = project_v_tile(c + 2)
        elif p < N_PAIRS:
            # KT[p] tail pieces early (deadline: own chunk 4*tt), then the
            # NEXT pair's QT + KT-tt0 (deadline: pair p+1 start).
            if c < 3:
                project_eT_tile(wk, p, c + 1, KT[p])
            elif p < N_PAIRS - 1 and c in (4, 6, 8, 10):
                project_eT_tile(wq, p + 1, (c - 4) // 2, QT[p + 1])
            elif p < N_PAIRS - 1 and c == 12:
                project_eT_tile(wk, p + 1, 0, KT[p + 1])

    for p in range(N_PAIRS):
        acc = [
            accps.tile([128, QB], FP, name=f"acc{qb}_{p}", tag=f"acc{qb}")
            for qb in range(2)
        ]
        pending_av = None
        for c in range(N_KC):
            kt_lo = KT[p][0:64, c * 128 : (c + 1) * 128]
            kt_hi = KT[p][64:128, c * 128 : (c + 1) * 128]

            def s_mm(dst, kt, base, q0):
                nc.tensor.matmul(
                    dst,
                    kt,
                    QT[p][base : base + 64, q0 : q0 + 512],
                    start=True,
                    stop=True,
                    tile_position=(base, 0),
                )

            def av_mm(hi, qb, qt, cc, vts_, pt_):
                nc.tensor.matmul(
                    acc[qb][:, qt * 512 : (qt + 1) * 512],
                    vts_[hi],
                    pt_[(hi, qb)][:, qt * 512 : (qt + 1) * 512],
                    start=(cc == 0 and hi == 0),
                    stop=(cc == N_KC - 1 and hi == 1),
                )

            # ---- scores qb0 (row-tiled concurrent pair) ----
            s0A = sps.tile([128, QB], FP, name=f"s_{p}_{c}_A0", tag="s")
            s0B = sps.tile([128, QB], FP, name=f"s_{p}_{c}_B0", tag="s")
            for qt in range(2):
                s_mm(s0A[:, qt * 512 : qt * 512 + 512], kt_lo, 0, qt * 512)
                s_mm(s0B[:, qt * 512 : qt * 512 + 512], kt_hi, 64, qt * 512)
            zs = zp.tile([128, 4], FP, name=f"zs_{p}_{c}", tag="zs")
            # head A qb0 -> Vector engine: Schraudolph int-convert (two halves
            # so s0A's PSUM buf frees as early as possible) + custom fixup.
            z0t = z0p.tile([128, QB], FP, name=f"z0_{p}_{c}", tag="z0")
            for h in range(2):
                nc.vector.tensor_scalar(
                    out=z0t.bitcast(I32)[:, h * 512 : (h + 1) * 512],
                    in0=s0A[:, h * 512 : (h + 1) * 512],
                    scalar1=A_SCHRAUD,
                    scalar2=B_SCHRAUD,
                    op0=mybir.AluOpType.mult,
                    op1=mybir.AluOpType.add,
                )
            pA0 = pp.tile([128, QB], BF, name=f"p_{p}_{c}_A0", tag="p")
            nc.vector._custom_dve(
                exp_op,
                out=pA0,
                in0=z0t,
                s0=MASK_VAL,
                s1=TWO_P126,
                imm2=QC,
                accum_out=zs[:, 0:1],
            )
            # head B qb0 -> Scalar engine
            pB0 = pp.tile([128, QB], BF, name=f"p_{p}_{c}_B0", tag="p")
            nc.scalar.activation(
                out=pB0, in_=s0B, func=Exp, scale=SCALE, accum_out=zs[:, 2:3]
            )
            # previous chunk's AV, first half (fills the PE while exp drains)
            if pending_av is not None:
                pc, pvts, ppt = pending_av
                for qt in range(2):
                    av_mm(0, 0, qt, pc, pvts, ppt)
                    av_mm(0, 1, qt, pc, pvts, ppt)
            # ---- scores qb1: B first (reuses s0A's buf, freed by op1) ----
            s1B = sps.tile([128, QB], FP, name=f"s_{p}_{c}_B1", tag="s")
            for qt in range(2):
                s_mm(s1B[:, qt * 512 : qt * 512 + 512], kt_hi, 64, QB + qt * 512)
            pB1 = pp.tile([128, QB], BF, name=f"p_{p}_{c}_B1", tag="p")
            nc.scalar.activation(
                out=pB1, in_=s1B, func=Exp, scale=SCALE, accum_out=zs[:, 3:4]
            )
            s1A = sps.tile([128, QB], FP, name=f"s_{p}_{c}_A1", tag="s")
            for qt in range(2):
                s_mm(s1A[:, qt * 512 : qt * 512 + 512], kt_lo, 0, QB + qt * 512)
            pA1 = pp.tile([128, QB], BF, name=f"p_{p}_{c}_A1", tag="p")
            nc.scalar.activation(
                out=pA1, in_=s1A, func=Exp, scale=SCALE, accum_out=zs[:, 1:2]
            )
            ptiles = {(0, 0): pA0, (0, 1): pA1, (1, 0): pB0, (1, 1): pB1}
            # previous chunk's AV, second half
            if pending_av is not None:
                pc, pvts, ppt = pending_av
                for qt in range(2):
                    av_mm(1, 0, qt, pc, pvts, ppt)
                    av_mm(1, 1, qt, pc, pvts, ppt)
            # ---- Z = qb0 + qb1 partial sums (gpsimd); r = 1/Z; V' = V*r ----
            za = zp.tile([128, 2], FP, name=f"za_{p}_{c}", tag="za")
            nc.gpsimd.tensor_add(za[:, 0:1], zs[:, 0:1], zs[:, 1:2])
            nc.gpsimd.tensor_add(za[:, 1:2], zs[:, 2:3], zs[:, 3:4])
            rz = zp.tile([128, 2], FP, name=f"rz_{p}_{c}", tag="rz")
            nc.vector.reciprocal(out=rz, in_=za)
            vts = []
            for hi in range(2):
                vt = vpp.tile([128, 128], BF, name=f"vp{hi}_{p}_{c}", tag=f"vp{hi}")
                lo, hi_ = (0, 64) if hi == 0 else (64, 128)
                zlo, zhi = (64, 128) if hi == 0 else (0, 64)
                nc.gpsimd.memset(vt[:, zlo:zhi], 0.0)
                nc.vector.tensor_scalar_mul(
                    vt[:, lo:hi_],
                    V[c][:, p * 128 + lo : p * 128 + hi_],
                    rz[:, hi : hi + 1],
                )
                vts.append(vt)
            pending_av = (c, vts, ptiles)
            # ---- fillers at slot end: their PSUM piece lands in the "s"
            # rotation after this chunk's 4 tiles; the copy runs early in the
            # next slot. ----
            emit_filler(p, c)
        pc, pvts, ppt = pending_av
        for hi in range(2):
            for qt in range(2):
                av_mm(hi, 0, qt, pc, pvts, ppt)
                av_mm(hi, 1, qt, pc, pvts, ppt)
        # epilogue: outT rows for this pair -> SBUF -> DRAM (host transposes)
        for qb in range(2):
            ot = op.tile([128, QB], FP, name=f"ot_{p}_{qb}", tag="ot")
            nc.vector.tensor_copy(ot, acc[qb])
            nc.sync.dma_start(
                out=outT[p * 128 : (p + 1) * 128, qb * QB : (qb + 1) * QB],
                in_=ot,
            )

    for pool in (op, vpp, z0p, zp, pp, accps, sps, qkv, wp, xw):
        pool.release()


def build():
    import concourse.bacc as bacc
    import concourse.mybir as mybir
    import concourse.tile as tile

    nc = bacc.Bacc("TRN2", target_bir_lowering=False, debug=False)
    FP = mybir.dt.float32
    FR = mybir.dt.float32r
    xT = nc.dram_tensor("xT", [D, T], FR, kind="ExternalInput").ap()
    wqT = nc.dram_tensor("wqT", [D, E], FR, kind="ExternalInput").ap()
    wkT = nc.dram_tensor("wkT", [D, E], FR, kind="ExternalInput").ap()
    wvT = nc.dram_tensor("wvT", [D, E], FR, kind="ExternalInput").ap()
    outT = nc.dram_tensor("outT", [E, T], FP, kind="ExternalOutput").ap()
    with tile.TileContext(nc) as tc:
        _build_kernel(tc, xT, wqT, wkT, wvT, outT)
    nc.compile()
    _split_multi_waits(nc)
    return nc


def _get_nc():
    global _built
    if _built is None:
        _built = build()
    return _built


def make_in_maps(x, Wq, Wk, Wv):
    in_maps = []
    for c in range(N_CORES):
        b, g = divmod(c, 2)
        e0 = E * g
        in_maps.append(
            {
                "xT": np.ascontiguousarray(x[b].T),
                "wqT": np.ascontiguousarray(Wq[e0 : e0 + E, :].T),
                "wkT": np.ascontiguousarray(Wk[e0 : e0 + E, :].T),
                "wvT": np.ascontiguousarray(Wv[e0 : e0 + E, :].T),
            }
        )
    return in_maps


def assemble_out(results):
    out = np.empty((B, T, D), np.float32)
    for c in range(N_CORES):
        b, g = divmod(c, 2)
        e0 = E * g
        out[b][:, e0 : e0 + E] = results[c]["outT"].T
    return out


def kernel(x, padding_mask, Wq, Wk, Wv):
    x = np.asarray(x, dtype=np.float32)
    padding_mask = np.asarray(padding_mask, dtype=np.float32)
    Wq = np.asarray(Wq, dtype=np.float32)
    Wk = np.asarray(Wk, dtype=np.float32)
    Wv = np.asarray(Wv, dtype=np.float32)
    if not np.all(padding_mask == 1.0):
        return _np_reference(x, padding_mask, Wq, Wk, Wv)

    from concourse.bass_utils import run_bass_kernel_spmd

    nc = _get_nc()
    in_maps = make_in_maps(x, Wq, Wk, Wv)
    res = run_bass_kernel_spmd(nc, in_maps, list(range(N_CORES)))
    return assemble_out(res.results)



# revision 31
# speedup vs baseline: 1.0969x; 1.0969x over previous
"""Multi-head self-attention (B=4, T=2048, D=1024, H=16) on 8 TRN2 NeuronCores.

Reference quirk: softmax normalizes over the QUERY axis (dim=2 of
[B,H,T1,T2]), i.e. attn[q,k] = exp(s[q,k]) / sum_q' exp(s[q',k]).

Sharding (fully SPMD, one NEFF for all 8 cores):
  core c -> batch b = c//2, head-group g = c%2 (8 heads = 512 cols of Wq/Wk/Wv).
  Host pre-slices AND pre-transposes per-core inputs (xT, wqT/wkT/wvT in
  bf16), runs the kernel, and stitches the 8 transposed [E, T] output shards.

v2 device algorithm per core (vs v1: concurrent PE tile-pairs):
  1. Projections in bf16 (same PE rate as fp32r, half the SBUF/DMA):
     QT/KT [128e, T] per head-pair, V [128t, 512e], via 8-dc PSUM-accumulated
     matmuls; PSUM->SBUF copies on VectorE (ScalarE during the ramp).
  2. Partition-swapped copies KSW/QSW (SBUF->SBUF DMA): head-A rows are
     duplicated into partitions 64-127 (and B into 0-63) so that BOTH
     row-tiles of the PE can compute the SAME head concurrently:
       SA tile [128k, 1024q] is written by MM(T0: KT[0:64], QT[0:64, qt0])
       and MM(T8: KSW[64:128], QSW[64:128, qt1]) -- measured dStart ~4ns
       (true row-tile concurrency, 2x score throughput).
  3. exp: single-head [128,1024] tiles; ScalarE ACT (accum_out -> Z) for
     ~2.5 tiles/chunk, VectorE Schraudolph (one-shot fp32->int32
     tensor_scalar + custom 7-stage DVE fixup op, accum_out) for ~1.5.
  4. AV with 2x col-tiling: V'A [128k,64] -> acc[0:64] (col grp 0) and
     V'B -> acc[64:128] (col grp 64) stream concurrently; V' = V * (1/Z)
     per key (gpsimd adds Z partials, VectorE reciprocal + V-scales).
     AV is deferred one chunk so it never waits on the Z chain.
  5. Epilogue: acc -> SBUF -> DRAM as outT [E, T]; final transpose on host.
"""

import operator

import numpy as np

B, T, D, H = 4, 2048, 1024, 16
DH = D // H
SCALE = 1.0 / (DH**0.5)
N_CORES = 8
E = D // 2  # 512 output cols per core (8 heads)
N_PAIRS = 4  # head-pairs per core
N_DC = D // 128  # 8 contraction chunks for projections
N_KC = T // 128  # 16 key chunks
QB = 1024  # exp free-dim block (2 PSUM banks)

# Schraudolph exp constants (int16/bf16 flavor): z16 = round(2^7 *
# (log2e*SCALE*s + 127)) stored as int16; its bits read as bf16 give
# 2^i*(1+f) with 7-bit f (bf16->fp32 input conversion in the DVE is
# value-preserving, so the fp32 mantissa is f<<16 and the same subnormal
# mask + 2^126 scale recovers f). The custom op multiplies by
# (1 + QC*f*(f-1)) to correct the mantissa interpolation; 16-bit in/out
# runs the DVE at 2x.
A_SCHRAUD = float((2.0**7) * np.log2(np.e) * SCALE)
B_SCHRAUD = 127.0 * (2.0**7)
MASK_VAL = float(np.int32(0x007FFFFF).view(np.float32))  # +subnormal mantissa mask
TWO_P126 = float(np.float32(2.0**126))
QC = 0.23547743862603948

_built = None  # (nc,) cache so repeat kernel() calls skip rebuild/recompile
_exp_op = None


def _np_reference(x, padding_mask, Wq, Wk, Wv):
    """Pure-numpy fallback, used only if the mask is not all-ones."""
    x64 = x.astype(np.float64)
    Q = (x64 @ Wq.T.astype(np.float64)).reshape(B, T, H, DH).transpose(0, 2, 1, 3)
    K = (x64 @ Wk.T.astype(np.float64)).reshape(B, T, H, DH).transpose(0, 2, 1, 3)
    V = (x64 @ Wv.T.astype(np.float64)).reshape(B, T, H, DH).transpose(0, 2, 1, 3)
    s = np.einsum("bhqd,bhkd->bhqk", Q, K) * SCALE
    s = np.where(padding_mask[:, None, :, :] == 0, -np.inf, s)
    s = s - s.max(axis=2, keepdims=True)
    p = np.exp(s)
    p = p / p.sum(axis=2, keepdims=True)
    out = np.einsum("bhqk,bhkd->bhqd", p, V)
    return out.transpose(0, 2, 1, 3).reshape(B, T, D).astype(np.float32)


def _get_exp_op():
    """Register (once) the custom DVE op: out = z0*(1 + QC*f*(f-1)) with
    f extracted from z0's mantissa bits, plus a running sum (accum_out)."""
    global _exp_op
    if _exp_op is not None:
        return _exp_op
    import concourse.dve_ops as dve_ops_mod
    from concourse.dve_ops import DveOp
    from concourse.dve_spec import AluOp, Bin, C0, C1, C2, One, Spec, Src0, Zero, lower
    from concourse.dve_uop import DveOpSpec

    _a = Bin(AluOp.BITWISE_AND, Src0, C0)  # mantissa bits as +subnormal
    _d = _a * C1  # f in [0,1)  (subnormal * 2^126)
    _e = _d - One
    _u = _d * _e
    _v = _u * C2
    _t = _v * Src0

    def _ref(in0, in1, c0, c1, c2):
        x = np.ascontiguousarray(np.asarray(in0, np.float32))
        mask = np.float32(c0 if np.isscalar(c0) else np.asarray(c0).ravel()[0])
        a_bits = x.view(np.int32) & mask.view(np.int32)
        f = a_bits.view(np.float32).astype(np.float64) * float(c1)
        out = x.astype(np.float64) * (1.0 + float(c2) * f * (f - 1.0))
        out = out.astype(np.float32)
        return out, out.sum(axis=-1, keepdims=True, dtype=np.float64)

    spec = Spec(body=Src0 + _t, accum=operator.add, accum_init=Zero, reference=_ref)

    name = "EXP_SFIX_ANT"
    existing = next((o for o in dve_ops_mod.OPS if o.name == name), None)
    if existing is not None:
        _exp_op = existing
        return existing
    sha = DveOpSpec(name=name, opcode=0, uops=lower(spec, ver="v3"), rd1_en=False).sha(
        "v3"
    )
    op = DveOp(name, spec, subdim=False, uops_sha={"v3": sha})
    dve_ops_mod.OPS.append(op)
    dve_ops_mod._SUB_OPCODE_FOR_NAME[name] = (
        dve_ops_mod._CUSTOM_DVE_ROW_BASE + len(dve_ops_mod.OPS) - 1
    )
    dve_ops_mod.CUSTOM_DVE_SPECS[name] = spec
    _exp_op = op
    return op


def _patch_missing_pe_waits(nc):
    """Tile's sync elision sometimes drops the PE-semaphore wait on a
    PSUM-reading instruction (seen when a score tile is written by a
    tile-positioned MM pair and the reader's only surviving wait is an
    unrelated WAR dep). Walk program order, track the PE completion count
    at each PSUM write, and the max PE wait each engine has carried; where
    a reader's requirement exceeds its engine's coverage, insert a
    single-wait drain right before it."""
    import concourse.mybir as mybir

    blocks = list(nc.m.functions[0].blocks)
    pe_sem = None  # (sync_type, id, ant_name)
    for blk in blocks:
        for inst in blk.instructions:
            si = inst.sync_info
            if si is None:
                continue
            for u in si.on_update:
                if u.ant_name and u.ant_name.startswith("PE"):
                    pe_sem = (u.sync_type, u.id, u.ant_name)
                    break
            if pe_sem:
                break
        if pe_sem:
            break
    assert pe_sem is not None, "no PE semaphore found"

    pe_count = 0
    last_writer = {}  # memref -> pe_count after writing MM completes
    cov = {}  # engine -> max PE wait carried so far
    n_patched = 0
    for blk in blocks:
        i = 0
        insts = blk.instructions
        while i < len(insts):
            inst = insts[i]
            si = inst.sync_info
            eng = inst.engine
            is_pe = str(eng).endswith("PE")
            if si is not None:
                for w in si.on_wait:
                    if w.ant_name == pe_sem[2] and w.wait_value is not None:
                        cov[eng] = max(cov.get(eng, 0), w.wait_value)
            if is_pe and type(inst).__name__ == "InstMatmult":
                inc = 0
                if si is not None:
                    for u in si.on_update:
                        if u.ant_name == pe_sem[2]:
                            inc += u.update_value or 0
                pe_count += inc
                for ap in inst.outs or []:
                    mr = getattr(ap, "memref", None)
                    if mr:
                        last_writer[str(mr)] = pe_count
            elif is_pe:
                if si is not None:
                    for u in si.on_update:
                        if u.ant_name == pe_sem[2]:
                            pe_count += u.update_value or 0
            else:
                need = 0
                for ap in list(inst.ins or []):
                    mr = getattr(ap, "memref", None)
                    if mr and str(mr) in last_writer:
                        need = max(need, last_writer[str(mr)])
                if need > cov.get(eng, 0):
                    w = mybir.SyncWait(
                        sync_type=pe_sem[0],
                        id=pe_sem[1],
                        wait_mode="sem-ge-imm",
                        ant_name=pe_sem[2],
                        wait_value=need,
                    )
                    d = mybir.InstDrain(
                        name=f"{inst.name}-pw",
                        engine=eng,
                        sync_info=mybir.SyncInfo(on_wait=[w], on_update=[]),
                    )
                    insts.insert(i, d)
                    n_patched += 1
                    cov[eng] = need
                    i += 1  # skip past the inserted drain
            i += 1
    if n_patched:
        import logging

        logging.getLogger(__name__).info(
            f"_patch_missing_pe_waits: inserted {n_patched} PE waits"
        )
    return n_patched


def _split_multi_waits(nc):
    """Walrus caps sync waits at 1 per instruction; Tile can emit several
    (tail drains, mid-stream event semaphores carrying an elided dep). Keep
    the last wait on the instruction and splice single-wait drains (same
    engine) IMMEDIATELY BEFORE it — semantically identical to the
    multi-wait, unlike appending to the previous block, which hoists the
    wait far too early (observed deadlocking/racing chunk-6 exp)."""
    import concourse.mybir as mybir

    blocks = list(nc.m.functions[0].blocks)
    for blk in blocks:
        insts = blk.instructions
        i = 0
        while i < len(insts):
            inst = insts[i]
            si = inst.sync_info
            if si is not None and si.on_wait and len(si.on_wait) > 1:
                waits = list(si.on_wait)
                keep, extra = waits[-1], waits[:-1]
                assert all(w.wait_mode == "sem-ge-imm" for w in extra), extra
                si.on_wait = [keep]
                for j, w in enumerate(extra):
                    d = mybir.InstEventSemaphore(
                        name=f"{inst.name}-ws{j}",
                        engine=inst.engine,
                        sync_info=mybir.SyncInfo(on_wait=[w], on_update=[]),
                    )
                    insts.insert(i, d)
                    i += 1
            i += 1


def _build_kernel(tc, xT, wqT, wkT, wvT, outT):
    import concourse.bass as bass  # noqa: F401
    import concourse.mybir as mybir

    nc = tc.nc
    FP = mybir.dt.float32
    BF = mybir.dt.bfloat16
    I32 = mybir.dt.int32
    I16 = mybir.dt.int16
    Exp = mybir.ActivationFunctionType.Exp
    exp_op = _get_exp_op()

    # long-lived pools
    # (lottery shift 1)
    xw = tc.alloc_tile_pool(name="xw", bufs=1)
    wp = tc.alloc_tile_pool(name="wp", bufs=3)
    qkv = tc.alloc_tile_pool(name="qkv", bufs=1)
    sw = tc.alloc_tile_pool(name="sw", bufs=1)
    # PSUM: S pool (2x [128,QB] = 4 banks) shared by scores and projections;
    # acc pool (2x [128,QB] = 4 banks) for outT accumulation.
    sps = tc.alloc_tile_pool(name="sps", bufs=2, space="PSUM")
    accps = tc.alloc_tile_pool(name="accps", bufs=1, space="PSUM")
    pp = tc.alloc_tile_pool(name="pp", bufs=12)
    zp = tc.alloc_tile_pool(name="zp", bufs=6)
    z0p = tc.alloc_tile_pool(name="z0p", bufs=3)
    vpp = tc.alloc_tile_pool(name="vpp", bufs=6)
    op = tc.alloc_tile_pool(name="op", bufs=2)

    # ---- loads ----
    # Weights first (small), then x in query-quarter-major order: the first
    # ramp pieces (KT0-tt0, QT0-tt0) need only quarter tt=0 of every xT chunk,
    # so they complete at ~25% of the x stream instead of after all of it.
    xTs = [None] * N_DC
    wq, wk, wv = [None] * N_DC, [None] * N_DC, [None] * N_DC
    for dc in range(N_DC):
        xTs[dc] = xw.tile([128, T], BF, name=f"xT{dc}", tag=f"x{dc}")
        for ws, wap, label in ((wq, wqT, "wq"), (wk, wkT, "wk"), (wv, wvT, "wv")):
            wt = wp.tile([128, E], BF, name=f"{label}{dc}", tag=f"w{dc}")
            nc.sync.dma_start(out=wt, in_=wap[dc * 128 : (dc + 1) * 128, :])
            ws[dc] = wt
    for tt in range(4):
        for dc in range(N_DC):
            nc.sync.dma_start(
                out=xTs[dc][:, tt * 512 : (tt + 1) * 512],
                in_=xT[dc * 128 : (dc + 1) * 128, tt * 512 : (tt + 1) * 512],
            )

    # ---- projection emitters (psum borrowed from the S pool tag) ----
    copy_flip = [0]
    ramp = [True]  # during the upfront ramp ScalarE is idle; share copies

    def _proj_copy(dst, src):
        """PSUM->SBUF projection copies on ScalarE: the DVE is the busier
        evacuation engine in steady state (converts + customs), and during
        the ramp ScalarE is idle anyway."""
        nc.scalar.copy(dst, src)
        copy_flip[0] += 1

    QT = [None] * N_PAIRS
    KT = [None] * N_PAIRS
    QSW = [None] * N_PAIRS
    KSW = [None] * N_PAIRS
    V = [None] * N_KC
    for pair in range(N_PAIRS):
        QT[pair] = qkv.tile([128, T], BF, name=f"qt{pair}", tag=f"qt{pair}")
        KT[pair] = qkv.tile([128, T], BF, name=f"kt{pair}", tag=f"kt{pair}")
        QSW[pair] = sw.tile([128, T], BF, name=f"qsw{pair}", tag=f"qsw{pair}")
        KSW[pair] = sw.tile([128, T], BF, name=f"ksw{pair}", tag=f"ksw{pair}")

    def project_eT_tile(ws, pair, tt, et, swt, q_mode):
        """One [128, 512] t-block of QT/KT pair tile `et`, then the
        partition-swapped SBUF->SBUF DMA copies into `swt`.

        q_mode: for QSW only blocks of one partition-half are consumed
        (lo-half needs blocks 0,2; hi-half blocks 1,3); KSW needs both."""
        ps = sps.tile([128, QB], FP, name=f"ps_{et.tensor.name}_{tt}", tag="s")
        for dc in range(N_DC):
            nc.tensor.matmul(
                ps[:, 0:512],
                ws[dc][:, pair * 128 : (pair + 1) * 128],
                xTs[dc][:, tt * 512 : (tt + 1) * 512],
                start=(dc == 0),
                stop=(dc == N_DC - 1),
            )
        blk = slice(tt * 512, (tt + 1) * 512)
        _proj_copy(et[:, blk], ps[:, 0:512])
        if q_mode:
            if tt % 2 == 0:  # B-copy into lo half (T0-path rhs for SB tiles)
                nc.sync.dma_start(out=swt[0:64, blk], in_=et[64:128, blk])
            else:  # A-copy into hi half (T8-path rhs for SA tiles)
                nc.sync.dma_start(out=swt[64:128, blk], in_=et[0:64, blk])
        else:
            nc.sync.dma_start(out=swt[0:64, blk], in_=et[64:128, blk])
            nc.sync.dma_start(out=swt[64:128, blk], in_=et[0:64, blk])

    def project_v_tile(tt):
        v = qkv.tile([128, E], BF, name=f"v{tt}", tag=f"v{tt}")
        ps = sps.tile([128, QB], FP, name=f"ps_v{tt}", tag="s")
        for dc in range(N_DC):
            nc.tensor.matmul(
                ps[:, 0:512],
                xTs[dc][:, tt * 128 : (tt + 1) * 128],
                wv[dc],
                start=(dc == 0),
                stop=(dc == N_DC - 1),
            )
        _proj_copy(v, ps[:, 0:512])
        return v

    # Ramp (DMA-gated): chunk-0-critical pieces first (KT0-tt0, QT0, V0),
    # then pair-1's early-deadline pieces. Late-deadline pieces (KT tails:
    # KT[p] piece tt isn't needed until pair p's chunk 4*tt) spread into the
    # chunk stream as PE fillers with per-pair schedules below.
    project_eT_tile(wk, 0, 0, KT[0], KSW[0], False)
    for tt in range(4):
        project_eT_tile(wq, 0, tt, QT[0], QSW[0], True)
    V[0] = project_v_tile(0)
    V[1] = project_v_tile(1)
    for tt in range(4):
        project_eT_tile(wq, 1, tt, QT[1], QSW[1], True)
    project_eT_tile(wk, 1, 0, KT[1], KSW[1], False)
    ramp[0] = False

    def emit_filler(p, c):
        if p == 0:
            if c < 3:  # KT0 piece tt (needed by chunk 4*tt)
                project_eT_tile(wk, 0, c + 1, KT[0], KSW[0], False)
            if c + 2 < N_KC:
                V[c + 2] = project_v_tile(c + 2)
        elif p < N_PAIRS:
            # KT[p] tail pieces early (deadline: own chunk 4*tt), then the
            # NEXT pair's QT + KT-tt0 (deadline: pair p+1 start).
            if c < 3:
                project_eT_tile(wk, p, c + 1, KT[p], KSW[p], False)
            elif p < N_PAIRS - 1 and c in (4, 6, 8, 10):
                project_eT_tile(wq, p + 1, (c - 4) // 2, QT[p + 1], QSW[p + 1], True)
            elif p < N_PAIRS - 1 and c == 12:
                project_eT_tile(wk, p + 1, 0, KT[p + 1], KSW[p + 1], False)

    for p in range(N_PAIRS):
        acc = [
            accps.tile([128, QB], FP, name=f"acc{qb}_{p}", tag=f"acc{qb}")
            for qb in range(2)
        ]
        pending_av = None
        for c in range(N_KC):
            ck = slice(c * 128, (c + 1) * 128)
            kA_lo = KT[p][0:64, ck]
            kA_hi = KSW[p][64:128, ck]
            kB_lo = KSW[p][0:64, ck]
            kB_hi = KT[p][64:128, ck]

            def s_pair(dst, k_lo, k_hi, q_lo_src, q_hi_src, qb):
                """One single-head [128,1024] score tile via a concurrent
                row-tiled MM pair: T0 does queries qb*1024..+512 (cols 0:512),
                T8 does +512..+1024 (cols 512:1024)."""
                q0 = qb * QB
                nc.tensor.matmul(
                    dst[:, 0:512],
                    k_lo,
                    q_lo_src[0:64, q0 : q0 + 512],
                    start=True,
                    stop=True,
                    tile_position=(0, 0),
                )
                nc.tensor.matmul(
                    dst[:, 512:1024],
                    k_hi,
                    q_hi_src[64:128, q0 + 512 : q0 + 1024],
                    start=True,
                    stop=True,
                    tile_position=(64, 0),
                )

            def av_pair(qb, qt, cc, vts_, pt_):
                """Col-tiled concurrent AV pair: head A -> acc rows 0:64
                (col grp 0), head B -> rows 64:128 (col grp 64)."""
                qs = slice(qt * 512, (qt + 1) * 512)
                nc.tensor.matmul(
                    acc[qb][0:64, qs],
                    vts_[0],
                    pt_[(0, qb)][:, qs],
                    start=(cc == 0),
                    stop=(cc == N_KC - 1),
                    tile_position=(0, 0),
                    skip_group_check=True,
                )
                nc.tensor.matmul(
                    acc[qb][64:128, qs],
                    vts_[1],
                    pt_[(1, qb)][:, qs],
                    start=(cc == 0),
                    stop=(cc == N_KC - 1),
                    tile_position=(0, 64),
                    skip_group_check=True,
                )

            # Evacuation engines ping-pong per PSUM buffer so each buffer's
            # two serial reads land on different engines and the DVE's
            # customs hide behind the Scalar reads. Buffer rotation is
            # SA0(b0), SB0(b1), SA1(b0), SB1(b1):
            #   even chunks: DVE, ACT, ACT, DVE   (DVE 2 tiles)
            #   odd  chunks: DVE, ACT, ACT, ACT   (DVE 1 tile; avg 1.5)
            sb1_on_dve = c % 2 == 0

            def conv_dve(src_ps, zcol):
                """Pass 1 of the DVE Schraudolph: PSUM fp32 -> int16 z."""
                z0t = z0p.tile([128, QB], BF, name=f"z0_{p}_{c}_{zcol}", tag="z0")
                nc.vector.tensor_scalar(
                    out=z0t.bitcast(I16),
                    in0=src_ps,
                    scalar1=A_SCHRAUD,
                    scalar2=B_SCHRAUD,
                    op0=mybir.AluOpType.mult,
                    op1=mybir.AluOpType.add,
                )
                return z0t

            def fix_dve(z0t, dst_p, zcol):
                """Pass 2: mantissa-corrected 2^z from the int16 bits."""
                nc.vector._custom_dve(
                    exp_op,
                    out=dst_p,
                    in0=z0t,
                    s0=MASK_VAL,
                    s1=TWO_P126,
                    imm2=QC,
                    accum_out=zs[:, zcol : zcol + 1],
                )

            def exp_act(src_ps, dst_p, zcol):
                nc.scalar.activation(
                    out=dst_p,
                    in_=src_ps,
                    func=Exp,
                    scale=SCALE,
                    accum_out=zs[:, zcol : zcol + 1],
                )

            zs = zp.tile([128, 4], FP, name=f"zs_{p}_{c}", tag="zs")
            # ---- scores qb0 ----
            sA0 = sps.tile([128, QB], FP, name=f"s_{p}_{c}_A0", tag="s")
            s_pair(sA0, kA_lo, kA_hi, QT[p], QSW[p], 0)
            sB0 = sps.tile([128, QB], FP, name=f"s_{p}_{c}_B0", tag="s")
            s_pair(sB0, kB_lo, kB_hi, QSW[p], QT[p], 0)
            pA0 = pp.tile([128, QB], BF, name=f"p_{p}_{c}_A0", tag="p")
            z0A = conv_dve(sA0, 0)
            pB0 = pp.tile([128, QB], BF, name=f"p_{p}_{c}_B0", tag="p")
            exp_act(sB0, pB0, 2)
            # previous chunk's AV, first half (fills the PE while exp drains)
            if pending_av is not None:
                pc, pvts, ppt = pending_av
                av_pair(0, 0, pc, pvts, ppt)
                av_pair(1, 0, pc, pvts, ppt)
            # ---- scores qb1 ----
            sA1 = sps.tile([128, QB], FP, name=f"s_{p}_{c}_A1", tag="s")
            s_pair(sA1, kA_lo, kA_hi, QT[p], QSW[p], 1)
            pA1 = pp.tile([128, QB], BF, name=f"p_{p}_{c}_A1", tag="p")
            exp_act(sA1, pA1, 1)
            sB1 = sps.tile([128, QB], FP, name=f"s_{p}_{c}_B1", tag="s")
            s_pair(sB1, kB_lo, kB_hi, QSW[p], QT[p], 1)
            pB1 = pp.tile([128, QB], BF, name=f"p_{p}_{c}_B1", tag="p")
            if sb1_on_dve:
                z0B = conv_dve(sB1, 3)
            else:
                exp_act(sB1, pB1, 3)
            # customs AFTER the converts so they never delay an evacuation
            fix_dve(z0A, pA0, 0)
            if sb1_on_dve:
                fix_dve(z0B, pB1, 3)
            ptiles = {(0, 0): pA0, (0, 1): pA1, (1, 0): pB0, (1, 1): pB1}
            # previous chunk's AV, second half
            if pending_av is not None:
                pc, pvts, ppt = pending_av
                av_pair(0, 1, pc, pvts, ppt)
                av_pair(1, 1, pc, pvts, ppt)
            # ---- Z = qb0 + qb1 partial sums (gpsimd); r = 1/Z; V' = V*r ----
            za = zp.tile([128, 2], FP, name=f"za_{p}_{c}", tag="za")
            nc.gpsimd.tensor_add(za[:, 0:1], zs[:, 0:1], zs[:, 1:2])
            nc.gpsimd.tensor_add(za[:, 1:2], zs[:, 2:3], zs[:, 3:4])
            rz = zp.tile([128, 2], FP, name=f"rz_{p}_{c}", tag="rz")
            nc.vector.reciprocal(out=rz, in_=za)
            vts = []
            for hi in range(2):
                vt = vpp.tile([128, 64], BF, name=f"vp{hi}_{p}_{c}", tag=f"vp{hi}")
                eng = nc.vector if hi == 0 else nc.gpsimd
                eng.tensor_scalar_mul(
                    vt,
                    V[c][:, p * 128 + hi * 64 : p * 128 + hi * 64 + 64],
                    rz[:, hi : hi + 1],
                )
                vts.append(vt)
            pending_av = (c, vts, ptiles)
            # ---- fillers at slot end ----
            emit_filler(p, c)
        pc, pvts, ppt = pending_av
        for qt in range(2):
            av_pair(0, qt, pc, pvts, ppt)
            av_pair(1, qt, pc, pvts, ppt)
        # epilogue: outT rows for this pair -> SBUF -> DRAM (host transposes)
        for qb in range(2):
            ot = op.tile([128, QB], FP, name=f"ot_{p}_{qb}", tag="ot")
            nc.vector.tensor_copy(ot, acc[qb])
            nc.sync.dma_start(
                out=outT[p * 128 : (p + 1) * 128, qb * QB : (qb + 1) * QB],
                in_=ot,
            )

    for pool in (op, vpp, z0p, zp, pp, accps, sps, sw, qkv, wp, xw):
        pool.release()


def build(for_sim=False):
    import concourse.bacc as bacc
    import concourse.mybir as mybir
    import concourse.tile as tile

    nc = bacc.Bacc("TRN2", target_bir_lowering=False, debug=False)
    FP = mybir.dt.float32
    BF = mybir.dt.bfloat16
    xT = nc.dram_tensor("xT", [D, T], BF, kind="ExternalInput").ap()
    wqT = nc.dram_tensor("wqT", [D, E], BF, kind="ExternalInput").ap()
    wkT = nc.dram_tensor("wkT", [D, E], BF, kind="ExternalInput").ap()
    wvT = nc.dram_tensor("wvT", [D, E], BF, kind="ExternalInput").ap()
    outT = nc.dram_tensor("outT", [E, T], FP, kind="ExternalOutput").ap()
    with tile.TileContext(nc) as tc:
        _build_kernel(tc, xT, wqT, wkT, wvT, outT)
    nc.compile()
    if not for_sim:
        # CoreSim handles multi-waits natively (and chokes on post-compile
        # instruction inserts); walrus needs them split to 1 wait/inst.
        _split_multi_waits(nc)
    return nc


def _get_nc():
    global _built
    if _built is None:
        _built = build()
    return _built


def make_in_maps(x, Wq, Wk, Wv):
    import ml_dtypes

    bf = ml_dtypes.bfloat16
    in_maps = []
    for c in range(N_CORES):
        b, g = divmod(c, 2)
        e0 = E * g
        in_maps.append(
            {
                "xT": np.ascontiguousarray(x[b].T).astype(bf),
                "wqT": np.ascontiguousarray(Wq[e0 : e0 + E, :].T).astype(bf),
                "wkT": np.ascontiguousarray(Wk[e0 : e0 + E, :].T).astype(bf),
                "wvT": np.ascontiguousarray(Wv[e0 : e0 + E, :].T).astype(bf),
            }
        )
    return in_maps


def assemble_out(results):
    out = np.empty((B, T, D), np.float32)
    for c in range(N_CORES):
        b, g = divmod(c, 2)
        e0 = E * g
        out[b][:, e0 : e0 + E] = results[c]["outT"].T
    return out


def kernel(x, padding_mask, Wq, Wk, Wv):
    x = np.asarray(x, dtype=np.float32)
    padding_mask = np.asarray(padding_mask, dtype=np.float32)
    Wq = np.asarray(Wq, dtype=np.float32)
    Wk = np.asarray(Wk, dtype=np.float32)
    Wv = np.asarray(Wv, dtype=np.float32)
    if not np.all(padding_mask == 1.0):
        return _np_reference(x, padding_mask, Wq, Wk, Wv)

    from concourse.bass_utils import run_bass_kernel_spmd

    nc = _get_nc()
    in_maps = make_in_maps(x, Wq, Wk, Wv)
    res = run_bass_kernel_spmd(nc, in_maps, list(range(N_CORES)))
    return assemble_out(res.results)
